# revision 20
# baseline (speedup 1.0000x reference)
"""Trainium2 kernel for: out = (mat1 @ mat2 + input_tensor).astype(f32), all int32 in [0,16).

Strategy
--------
Values are integers in [0, 15], so:
  - mat1/mat2 are exact in fp8 e4m3 (integers 0..15 need 4 significand bits; e4m3 has 4)
  - products (<= 225) are exact in the PE datapath (e6m3 upcast -> e10m10 product)
  - accumulators (<= 15*15*4096 + 15 = 921,615 < 2^24) are exact in fp32 PSUM
so an fp8 DoubleRow matmul (2 MACs/cell/cycle, the fastest PE mode on trn2)
reproduces the int32 reference bit-exactly in fp32.

Sharding: 2D, 4 mat1-row blocks x 2 mat2-column blocks over 8 cores. Each
core computes a [1024, 2048] slab of the output. Pure SPMD, no collectives.

The MM stream runs at the fp8 peak (216 ns issue-to-issue per
[K=256]x[128,512] DoubleRow matmul; 512 MMs = 110.6 us); the rest is
overhead engineering, driven by trace measurements:
  - The DMA backend ramps slowly for the first ~5-6 us after the NEFF
    preamble (packet processing starts ~5x slow), so the supply side is a
    single strictly-FIFO Sync-ring queue ordered EXACTLY by first-use:
    m2/m1 phase-a chunks interleaved in consumption order, then phase-b,
    then next-block prefetches. Per-DMA completion sems then release
    compute in lockstep with arrival.
  - Output stores ride the otherwise-idle Scalar HW-DGE ring so a pending
    store can never head-of-line-block a load.
  - ~50 PE-warmup matmuls on a zeroed tile bridge the DMA ramp: they
    release the HAM clock-gate (1.2 -> 2.4 GHz after ~3.4 us of sustained
    PE activity) and are sized to end just after the head-critical data
    lands -- the PE must never idle >~1 us or the HAM re-throttles
    (costs ~1.7 us per oscillation).
  - Instruction count is minimized (~35 DMAs vs 59 in the v1 baseline;
    DMA-instruction issue costs ~0.6 us of HW-DGE ring time each): PSUM
    evicted in 2-bank pairs, one batched output store per n-block except
    the last, which drains per-m-tile -- the final 2-bank pair in four
    chunks with stores alternated across both rings -- so the exit tail
    after the last matmul stays ~3 us.
  - NOTE (measured, not controllable from the kernel): the NEFF epilogue
    resets all 249 HW semaphores at ~120 ns each split across the 5
    engine queues (~6.2 us) plus barrier/drain choreography (~2 us), and
    the DMA backend needs ~5 us of ramp before the first ~1 MiB lands.
    Those ~13 us are fixed overhead for any kernel under this runtime.

Per-core device program:
  - mat1 resident in one SBUF tile (fp8, DoubleRow-interleaved on host),
    loaded as 8 per-mt phase-a halves + 3 phase-b chunks
  - mat2 streamed one n-block (512 cols, all of K) per DMA, double-buffered;
    nb0 split in 5 chunks so compute starts as each lands
  - 16 DoubleRow matmuls (K=256 each) accumulate a [128, 512] fp32 PSUM
    bank; 8 m-tiles use all 8 PSUM banks per n-block; nb0 contracts in two
    kt-phases so the PE never outruns the initial DMA ramp
  - DVE adds the (fp8) input_tensor during PSUM->SBUF eviction, two banks
    at a time
"""

import numpy as np
import ml_dtypes

import concourse.bass as bass
import concourse.mybir as mybir
import concourse.tile as tile
from concourse import bacc
from concourse.bass_utils import run_bass_kernel_spmd

F8 = mybir.dt.float8e4
F32 = mybir.dt.float32

N_CORES = 8
A_SHARD = 4  # mat1 row blocks
B_SHARD = 2  # mat2 col blocks
P = 128  # partitions
NB_TILE = 512  # output free-dim tile (one PSUM bank of fp32)
KP = 256  # contraction per DoubleRow matmul (2 x 128)
N_WARMUP = 54


def build_program(m_shard: int, K: int, n_shard: int) -> bass.Bass:
    """One NeuronCore's program: [m_shard, K] @ [K, n_shard] + input -> fp32.

    DRAM parameter layouts (host pre-packs; p is the SBUF partition index):
      m1a [MT, P, KT/2, 2, P] fp8  : m1a[mt, p, k, i, m] = mat1_blk[P*mt + m, KP*k + 128*i + p]
      m1b [P, MT, KT/2, 2, P] fp8  : m1b[p, mt, k, i, m] = mat1_blk[P*mt + m, KP*(KT/2 + k) + 128*i + p]
      m2  [NB, P, KT, 2, 512] fp8  : m2[nb, p, kt, i, n] = mat2_blk[KP*kt + 128*i + p, 512*nb + n]
      inp [NB, P, MT, 512] fp8     : inp[nb, p, mt, n] = input_blk[P*mt + p, 512*nb + n]
      out [NB, P, MT, 512] f32     : out[nb, p, mt, n] = result[P*mt + p, 512*nb + n]
    """
    KT = K // KP
    KH = KT // 2
    MT = m_shard // P
    NB = n_shard // NB_TILE

    nc = bacc.Bacc("TRN2", target_bir_lowering=False, debug=False)
    m1ad = nc.dram_tensor("m1a", [MT, P, KH, 2, P], F8, kind="ExternalInput")
    m1bd = nc.dram_tensor("m1b", [P, MT, KH, 2, P], F8, kind="ExternalInput")
    m2d = nc.dram_tensor("m2", [NB, P, KT, 2, NB_TILE], F8, kind="ExternalInput")
    inpd = nc.dram_tensor("inp", [NB, P, MT, NB_TILE], F8, kind="ExternalInput")
    outd = nc.dram_tensor("out", [NB, P, MT, NB_TILE], F32, kind="ExternalOutput")

    with tile.TileContext(nc) as tc:
        with (
            tc.tile_pool(name="m1", bufs=1) as m1_pool,
            tc.tile_pool(name="m2", bufs=2) as m2_pool,
            tc.tile_pool(name="inp", bufs=2) as inp_pool,
            tc.tile_pool(name="res", bufs=2) as res_pool,
            # 3 two-bank pairs (mt0-5) + 2 single banks (mt6, mt7): the
            # singles decouple the last m-tiles' eviction from each other,
            # so only mt7's drain trails the final matmul group.
            tc.tile_pool(name="psum", bufs=3, space="PSUM") as psum_pool,
            tc.tile_pool(name="psum1", bufs=2, space="PSUM") as psum1_pool,
        ):
            # mat1 lives in one SBUF tile for the whole run.
            m1s = m1_pool.tile([P, MT, KT, 2, P], F8, name="m1", tag="m1", bufs=1)

            m2_tiles = {}
            inp_tiles = {}
            m2_tiles[0] = m2_pool.tile([P, KT, 2, NB_TILE], F8, tag="m2", name="m2s0")
            inp_tiles[0] = inp_pool.tile([P, MT, NB_TILE], F8, tag="inp", name="inp0")

            # Sync HW-DGE ring, strict FIFO: every transfer is queued in
            # first-use order of the nb0 two-phase sweep, so completions
            # release matmuls in lockstep with the DMA ramp.
            nc.sync.dma_start(m2_tiles[0][:, 0:2], m2d[0, :, 0:2])
            nc.sync.dma_start(m1s[:, 0, :KH], m1ad[0])
            nc.sync.dma_start(m2_tiles[0][:, 2:4], m2d[0, :, 2:4])
            nc.sync.dma_start(m2_tiles[0][:, 4:6], m2d[0, :, 4:6])
            nc.sync.dma_start(m2_tiles[0][:, 6:8], m2d[0, :, 6:8])
            for mt in range(1, 4):
                nc.sync.dma_start(m1s[:, mt, :KH], m1ad[mt])
            nc.sync.dma_start(m2_tiles[0][:, 8:16], m2d[0, :, 8:16])
            for mt in range(4, MT):
                nc.sync.dma_start(m1s[:, mt, :KH], m1ad[mt])
            # phase-b mat1 halves, in phase-b consumption order
            nc.sync.dma_start(m1s[:, 0:2, KH:], m1bd[:, 0:2])
            nc.sync.dma_start(inp_tiles[0][:], inpd[0])
            nc.sync.dma_start(m1s[:, 2:4, KH:], m1bd[:, 2:4])
            nc.sync.dma_start(m1s[:, 4:MT, KH:], m1bd[:, 4:MT])
            if NB > 1:
                m2_tiles[1] = m2_pool.tile(
                    [P, KT, 2, NB_TILE], F8, tag="m2", name="m2s1"
                )
                nc.sync.dma_start(m2_tiles[1][:], m2d[1])
                inp_tiles[1] = inp_pool.tile(
                    [P, MT, NB_TILE], F8, tag="inp", name="inp1"
                )
                nc.sync.dma_start(inp_tiles[1][:], inpd[1])

            # PE warmup on a zeroed tile while the DMAs stream in: burns the
            # HAM cold window (~3.4us at 1.2 GHz) during the DMA head so the
            # real matmuls run at 2.4 GHz from the start. Slight overshoot is
            # intentional -- a post-warmup idle gap re-throttles the PE.
            warm_src = inp_pool.tile([P, P], F8, tag="warm", bufs=1)
            nc.gpsimd.memset(warm_src[:], 0.0)
            warm_ps = psum_pool.tile([P, 2, NB_TILE], F32, tag="ps")
            for _ in range(N_WARMUP):
                nc.tensor.matmul(
                    warm_ps[:, 0, :P], warm_src[:], warm_src[:], start=True, stop=True
                )

            for nb in range(NB):
                if nb + 2 < NB:
                    # Issue sits in the Sync queue until the m2/inp buffers
                    # free up (their consumers are two nbs back).
                    m2_tiles[nb + 2] = m2_pool.tile(
                        [P, KT, 2, NB_TILE], F8, tag="m2", name=f"m2s{nb + 2}"
                    )
                    nc.sync.dma_start(m2_tiles[nb + 2][:], m2d[nb + 2])
                    inp_tiles[nb + 2] = inp_pool.tile(
                        [P, MT, NB_TILE], F8, tag="inp", name=f"inp{nb + 2}"
                    )
                    nc.sync.dma_start(inp_tiles[nb + 2][:], inpd[nb + 2])
                m2s = m2_tiles.pop(nb)
                inps = inp_tiles.pop(nb)
                outs = res_pool.tile([P, MT, NB_TILE], F32)
                # nb=0 runs its contraction in two kt-phases across all
                # m-tiles: compute on the first half of m2 starts while the
                # second half is still in flight.
                phases = [(0, KH), (KH, KT)] if nb == 0 else [(0, KT)]
                pss = [
                    psum_pool.tile([P, 2, NB_TILE], F32, name=f"ps_{pr}", tag="ps")
                    for pr in range(MT // 2 - 1)
                ]
                sgl = [
                    psum1_pool.tile([P, NB_TILE], F32, name=f"ps1_{i}", tag="ps1")
                    for i in range(2)
                ]

                def bank(mt):
                    return pss[mt // 2][:, mt % 2] if mt < MT - 2 else sgl[mt - (MT - 2)][:]

                last = nb == NB - 1
                for k0, k1 in phases:
                    for mt in range(MT):
                        for kt in range(k0, k1):
                            nc.tensor.matmul(
                                bank(mt),
                                m1s[:, mt, kt],
                                m2s[:, kt],
                                start=(kt == 0),
                                stop=(kt == KT - 1),
                                perf_mode=mybir.MatmulPerfMode.DoubleRow,
                            )
                if not last:
                    for pr in range(MT // 2 - 1):
                        nc.vector.tensor_add(
                            outs[:, 2 * pr : 2 * pr + 2],
                            pss[pr][:],
                            inps[:, 2 * pr : 2 * pr + 2],
                        )
                    for i in range(2):
                        mt = MT - 2 + i
                        nc.vector.tensor_add(outs[:, mt], sgl[i][:], inps[:, mt])
                    # One batched 2 MiB store per n-block, on the Scalar ring.
                    nc.scalar.dma_start(outd[nb], outs[:])
                else:
                    # Last n-block: per-mt adds/stores so results drain as
                    # they complete (mt_even waits its pair partner's group;
                    # mt6 is a single bank so it drains mid-stream too).
                    # Only mt7 trails the final matmul group: drained as a
                    # 384+128 split -- the short add releases the last store
                    # early and the final payload is small -- with the two
                    # stores on different HW-DGE rings.
                    for mt in range(MT - 1):
                        nc.vector.tensor_add(outs[:, mt], bank(mt), inps[:, mt])
                        nc.scalar.dma_start(outd[nb, :, mt], outs[:, mt])
                    mt = MT - 1
                    for hs, eng in ((slice(0, 448), nc.sync),
                                    (slice(448, NB_TILE), nc.scalar)):
                        nc.vector.tensor_add(
                            outs[:, mt, hs], sgl[1][:, hs], inps[:, mt, hs]
                        )
                        eng.dma_start(outd[nb, :, mt, hs], outs[:, mt, hs])
    nc.compile()
    return nc


def pack_m1_block(blk: np.ndarray) -> tuple[np.ndarray, np.ndarray]:
    """[m_shard, K] int -> (m1a [MT,P,KH,2,P], m1b [P,MT,KH,2,P]) fp8."""
    m_shard, K = blk.shape
    KT = K // KP
    KH = KT // 2
    # [mt, m, kt, i, p] from blk[P*mt + m, KP*kt + 128*i + p]
    r = blk.reshape(m_shard // P, P, KT, 2, P)
    f8 = r.astype(np.float32).astype(ml_dtypes.float8_e4m3)
    m1a = np.ascontiguousarray(f8[:, :, :KH].transpose(0, 4, 2, 3, 1))
    m1b = np.ascontiguousarray(f8[:, :, KH:].transpose(4, 0, 2, 3, 1))
    return m1a, m1b


def pack_m2(mat2: np.ndarray) -> np.ndarray:
    """[K, N] int -> [N//512, P, KT, 2, 512] fp8 (DoubleRow moving layout)."""
    K, N = mat2.shape
    r = mat2.reshape(K // KP, 2, P, N // NB_TILE, NB_TILE)  # [kt, i, p, nb, n]
    return np.ascontiguousarray(r.transpose(3, 2, 0, 1, 4)).astype(np.float32).astype(
        ml_dtypes.float8_e4m3
    )


def pack_inp_block(blk: np.ndarray) -> np.ndarray:
    """[m_shard, n_shard] int -> [NB, P, MT, 512] fp8 (0..15 are exact)."""
    m_shard, n_shard = blk.shape
    r = blk.reshape(m_shard // P, P, n_shard // NB_TILE, NB_TILE)  # [mt, p, nb, n]
    return (
        np.ascontiguousarray(r.transpose(2, 1, 0, 3))
        .astype(np.float32)
        .astype(ml_dtypes.float8_e4m3)
    )


def unpack_out(packed: np.ndarray, m_shard: int, n_shard: int) -> np.ndarray:
    """[NB, P, MT, 512] f32 -> [m_shard, n_shard] f32."""
    return np.ascontiguousarray(packed.transpose(2, 1, 0, 3)).reshape(m_shard, n_shard)


def _prepare(input_tensor, mat1, mat2):
    input_tensor = np.asarray(input_tensor)
    mat1 = np.asarray(mat1)
    mat2 = np.asarray(mat2)
    M, K = mat1.shape
    N = mat2.shape[1]
    m_shard = M // A_SHARD
    n_shard = N // B_SHARD
    nb_per_core = n_shard // NB_TILE

    nc = build_program(m_shard, K, n_shard)

    m2p = pack_m2(mat2)  # [N//512, P, KT, 2, 512]; core takes its nb range
    in_maps = []
    for c in range(N_CORES):
        ra, cb = divmod(c, B_SHARD)
        rows = slice(ra * m_shard, (ra + 1) * m_shard)
        cols = slice(cb * n_shard, (cb + 1) * n_shard)
        nbs = slice(cb * nb_per_core, (cb + 1) * nb_per_core)
        m1a, m1b = pack_m1_block(mat1[rows])
        in_maps.append(
            {
                "m1a": m1a,
                "m1b": m1b,
                "m2": m2p[nbs],
                "inp": pack_inp_block(input_tensor[rows, cols]),
            }
        )
    return nc, in_maps, (m_shard, n_shard)


def _gather(results, m_shard, n_shard):
    M = m_shard * A_SHARD
    N = n_shard * B_SHARD
    out = np.empty((M, N), dtype=np.float32)
    for c in range(N_CORES):
        ra, cb = divmod(c, B_SHARD)
        out[
            ra * m_shard : (ra + 1) * m_shard, cb * n_shard : (cb + 1) * n_shard
        ] = unpack_out(results[c]["out"], m_shard, n_shard)
    return out


def kernel(input_tensor, mat1, mat2):
    nc, in_maps, (m_shard, n_shard) = _prepare(input_tensor, mat1, mat2)
    res = run_bass_kernel_spmd(nc, in_maps, list(range(N_CORES))).results
    return _gather(res, m_shard, n_shard)


def kernel_traced(input_tensor, mat1, mat2, **kwargs):
    """Like kernel(), but also returns BassKernelResults (exec_time_ns etc.)."""
    nc, in_maps, (m_shard, n_shard) = _prepare(input_tensor, mat1, mat2)
    res = run_bass_kernel_spmd(
        nc, in_maps, list(range(N_CORES)), trace=True, **kwargs
    )
    return _gather(res.results, m_shard, n_shard), res
